# revision 10
# baseline (speedup 1.0000x reference)
"""ArcFace loss_fn kernel for Trainium2, 8 NeuronCores.

Problem (hardcoded shapes): x [1024, 512] f32, W [100000, 512] f32,
b [100000] f32, labels [1024] int. Returns (prediction [1024, 100000] f32,
loss scalar f32) matching:

    Wn = W / max(||W_row||, 1e-12)
    wf = x @ Wn.T + b
    prediction = softmax(wf, axis=1)
    wf_m = cos(arccos(clip(wf, -1+eps, 1-eps)) + 0.1)
    numerator = 20 * wf_m[i, y_i]
    excl = sum_j exp(20*wf[i,j]) - exp(20*wf[i,y_i])
    loss = -mean(numerator - log(exp(numerator) + excl))

Sharding: class dim C split over 8 cores (12500 classes/core, zero-padded to
12544 = 98*128). Each core holds W.T + W shards, computes local logits via f32
TensorE matmuls, exp via ScalarE (fused scale=1/||W||, bias=b per-partition),
class-sums via DVE accumulate + ones-matmul partition fold, one small
AllReduce per batch-group for the global softmax denominator, and writes its
normalized prediction shard. The O(B)-sized loss assembly (target gather via
pred[i, y_i], cos/arccos, final log) runs on host in f32 — all O(B*C) work is
on device.
"""

import numpy as np

import concourse.bass as bass
import concourse.mybir as mybir
import concourse.tile as tile
from concourse import bacc
from concourse.bass_utils import run_bass_kernel_spmd

# problem constants
B = 1024
D = 512
C = 100000
S = 20.0
MARGIN = 0.1
EPS = 1e-7
NC = 8

CL = C // NC          # 12500 classes per core
P = 128
NT = 98               # C-tiles per core (12544 / 128)
CLP = NT * P          # 12544 padded classes per core
KT = D // P           # 4 contraction slices
B_PAD = -10000.0      # bias for padded classes -> exp(...) == 0
EXP_SHIFT = 100.0     # device computes exp(S*wf - EXP_SHIFT) so sums stay finite;
                      # host rescales by e^EXP_SHIFT in f32 to recover IEEE inf

# batch groups: E shard for one group must fit in SBUF alongside W stream
GROUPS = [(0, 344), (344, 340), (684, 340)]
BG_MAX = 344

F32 = mybir.dt.float32
BF16 = mybir.dt.bfloat16
SLABW = 512           # classes per W slab; hi|lo packed as [128, 2*SLABW] bf16
NSLAB = (CLP + SLABW - 1) // SLABW
AF = mybir.ActivationFunctionType
ALU = mybir.AluOpType

_CACHE = {}


def _build():
    nc = bacc.Bacc("TRN2", target_bir_lowering=False, debug=False, num_devices=NC)

    xt_d = nc.dram_tensor("xt", [D, B], F32, kind="ExternalInput").ap()
    wt_d = nc.dram_tensor("wt", [D, CLP], F32, kind="ExternalInput").ap()
    wn_d = nc.dram_tensor("wn", [CLP, D], F32, kind="ExternalInput").ap()
    b_d = nc.dram_tensor("bv", [CLP], F32, kind="ExternalInput").ap()
    out_d = nc.dram_tensor("out", [CLP, B], F32, kind="ExternalOutput").ap()
    stats_d = nc.dram_tensor("stats", [2, B], F32, kind="ExternalOutput").ap()

    with tile.TileContext(nc) as tc:
        with (
            tc.tile_pool(name="persist", bufs=1) as persist,
            tc.tile_pool(name="ebuf", bufs=1) as ebuf,
            tc.tile_pool(name="wtp", bufs=2) as wtp,
            tc.tile_pool(name="wtfp", bufs=4) as wtfp,
            tc.tile_pool(name="wnp", bufs=2) as wnp,
            tc.tile_pool(name="work", bufs=1) as work,
            tc.tile_pool(name="e20p", bufs=2) as e20p,
            tc.tile_pool(name="psmm", bufs=4, space="PSUM") as psmm,
            tc.tile_pool(name="psfold", bufs=1, space="PSUM") as psfold,
            tc.tile_pool(name="psbc", bufs=1, space="PSUM") as psbc,
            tc.tile_pool(name="psdump", bufs=1, space="PSUM") as psdump,
            tc.tile_pool(name="dram", bufs=1, space="DRAM") as dram,
        ):
            # ---- resident small tensors ----
            # x split into bf16 hi + lo (x = hi + lo to ~2^-17 rel)
            xhi = [
                persist.tile([P, B], BF16, tag=f"xhi{k}", name=f"xhi{k}")
                for k in range(KT)
            ]
            xlo = [
                persist.tile([P, B], BF16, tag=f"xlo{k}", name=f"xlo{k}")
                for k in range(KT)
            ]
            for k in range(KT):
                xtf = wtfp.tile([P, B], F32, tag="xtf", name="xtf", bufs=1)
                nc.sync.dma_start(xtf[:], xt_d[k * P : (k + 1) * P, :])
                nc.vector.tensor_copy(xhi[k][:], xtf[:])
                nc.vector.scalar_tensor_tensor(
                    xlo[k][:], xhi[k][:], -1.0, xtf[:], ALU.mult, ALU.add
                )

            ones_p = persist.tile([P, 1], F32, tag="ones_p")
            nc.vector.memset(ones_p[:], 1.0)
            ones_k1 = persist.tile([1, P], F32, tag="ones_k1")
            nc.vector.memset(ones_k1[:], 1.0)

            b_all = persist.tile([P, NT], F32, tag="b_all")
            nc.sync.dma_start(b_all[:], b_d.rearrange("(t p) -> p t", p=P))
            b20 = persist.tile([P, NT], F32, tag="b20")
            nc.vector.tensor_scalar(b20[:], b_all[:], S, -EXP_SHIFT, ALU.mult, ALU.add)

            # DRAM spill for bf16 hi|lo W blocks, written during group 0
            wt_hl_d = dram.tile(
                [NSLAB, KT, P, 2 * SLABW], BF16, tag="wt_hl", name="wt_hl"
            )

            # ---- W row norms: ssq per class via Square + accum ----
            # chunked so early C-tiles' inv/inv20 unblock before the full pass
            ssq = persist.tile([P, NT], F32, tag="ssq")
            ssq_c = persist.tile([P, NT], F32, tag="ssq_c")
            inv = persist.tile([P, NT], F32, tag="inv")
            inv20 = persist.tile([P, NT], F32, tag="inv20")
            zz = persist.tile([P, NT], F32, tag="zz")
            nrm = persist.tile([P, NT], F32, tag="nrm")
            NCHUNK = 25
            for c0 in range(0, NT, NCHUNK):
                c1 = min(c0 + NCHUNK, NT)
                for t in range(c0, c1):
                    wn_t = wnp.tile([P, D], F32, tag="wn", name="wn_t")
                    nc.sync.dma_start(wn_t[:], wn_d[t * P : (t + 1) * P, :])
                    dump = psdump.tile([P, D], F32, tag="dump", name="dump")
                    nc.scalar.activation(
                        dump[:], wn_t[:], AF.Square, accum_out=ssq[:, t : t + 1]
                    )
                cs = slice(c0, c1)
                # inv = 1/sqrt(max(ssq,1e-24)), two Newton steps for f32 accuracy
                nc.vector.tensor_scalar_max(ssq_c[:, cs], ssq[:, cs], 1e-24)
                nc.scalar.activation(nrm[:, cs], ssq_c[:, cs], AF.Sqrt)
                nc.vector.reciprocal(inv[:, cs], nrm[:, cs])
                for _ in range(2):
                    nc.vector.tensor_mul(zz[:, cs], inv[:, cs], inv[:, cs])
                    nc.vector.tensor_mul(zz[:, cs], zz[:, cs], ssq_c[:, cs])
                    nc.vector.tensor_scalar(
                        zz[:, cs], zz[:, cs], -0.5, 1.5, ALU.mult, ALU.add
                    )
                    nc.vector.tensor_mul(inv[:, cs], inv[:, cs], zz[:, cs])
                nc.vector.tensor_scalar_mul(inv20[:, cs], inv[:, cs], S)

            # ---- main: per batch-group ----
            for gi, (g0, bg) in enumerate(GROUPS):
                accE = persist.tile([P, BG_MAX], F32, tag="accE")
                accE20 = persist.tile([P, BG_MAX], F32, tag="accE20")
                nc.vector.memset(accE[:, :bg], 0.0)
                nc.vector.memset(accE20[:, :bg], 0.0)

                e_tiles = []
                for si, t0 in enumerate(range(0, NT, 4)):
                    nts = min(4, NT - t0)
                    w = nts * P
                    wts = []
                    for k in range(KT):
                        hl = wtp.tile(
                            [P, 2 * SLABW], BF16, tag=f"wt{k}", name=f"wt{k}"
                        )
                        if gi == 0:
                            # split W.T slab into bf16 hi|lo and spill to DRAM
                            wtf = wtfp.tile([P, SLABW], F32, tag="wtf", name="wtf")
                            nc.sync.dma_start(
                                wtf[:, :w],
                                wt_d[k * P : (k + 1) * P, t0 * P : t0 * P + w],
                            )
                            nc.scalar.activation(hl[:, :w], wtf[:, :w], AF.Copy)
                            nc.vector.scalar_tensor_tensor(
                                hl[:, SLABW : SLABW + w],
                                hl[:, :w],
                                -1.0,
                                wtf[:, :w],
                                ALU.mult,
                                ALU.add,
                            )
                            nc.sync.dma_start(
                                wt_hl_d[si, k, :, : SLABW + w], hl[:, : SLABW + w]
                            )
                        else:
                            nc.sync.dma_start(hl[:], wt_hl_d[si, k, :, :])
                        wts.append(hl)
                    for j in range(nts):
                        t = t0 + j
                        ps = psmm.tile([P, BG_MAX], F32, name="ps")
                        mm = 0
                        for k in range(KT):
                            hi = wts[k][:, j * P : (j + 1) * P]
                            lo = wts[k][:, SLABW + j * P : SLABW + (j + 1) * P]
                            for wop, xop in ((hi, xhi[k]), (hi, xlo[k]), (lo, xhi[k])):
                                nc.tensor.matmul(
                                    ps[:, :bg],
                                    wop,
                                    xop[:, g0 : g0 + bg],
                                    start=(mm == 0),
                                    stop=(mm == 3 * KT - 1),
                                )
                                mm += 1
                        e_t = ebuf.tile(
                            [P, BG_MAX], F32, tag=f"e{t}", name=f"e{t}",
                            bufs=2 if t < 8 else 1,
                        )
                        nc.scalar.activation(
                            e_t[:, :bg],
                            ps[:, :bg],
                            AF.Exp,
                            bias=b_all[:, t : t + 1],
                            scale=inv[:, t : t + 1],
                        )
                        e20_t = e20p.tile([P, BG_MAX], F32, tag="e20", name="e20")
                        nc.scalar.activation(
                            e20_t[:, :bg],
                            ps[:, :bg],
                            AF.Exp,
                            bias=b20[:, t : t + 1],
                            scale=inv20[:, t : t + 1],
                        )
                        nc.vector.tensor_add(accE[:, :bg], accE[:, :bg], e_t[:, :bg])
                        nc.vector.tensor_add(
                            accE20[:, :bg], accE20[:, :bg], e20_t[:, :bg]
                        )
                        e_tiles.append(e_t)

                # fold partitions: [128, bg] -> [1, bg]
                psE = psfold.tile([1, BG_MAX], F32, tag="psE")
                nc.tensor.matmul(
                    psE[:, :bg], ones_p[:], accE[:, :bg], start=True, stop=True
                )
                sumE = work.tile([1, BG_MAX], F32, tag="sumE")
                nc.vector.tensor_copy(sumE[:, :bg], psE[:, :bg])

                psE20 = psfold.tile([1, BG_MAX], F32, tag="psE20")
                nc.tensor.matmul(
                    psE20[:, :bg], ones_p[:], accE20[:, :bg], start=True, stop=True
                )
                sumE20 = work.tile([1, BG_MAX], F32, tag="sumE20")
                nc.vector.tensor_copy(sumE20[:, :bg], psE20[:, :bg])
                nc.sync.dma_start(stats_d[1:2, g0 : g0 + bg], sumE20[:, :bg])

                # AllReduce softmax denominator across the 8 cores
                cc_in = dram.tile([1, bg], F32, tag=f"ccin{gi}")
                cc_out = dram.tile([1, bg], F32, tag=f"ccout{gi}")
                nc.sync.dma_start(cc_in[:], sumE[:, :bg])
                nc.gpsimd.collective_compute(
                    "AllReduce",
                    ALU.add,
                    replica_groups=[list(range(NC))],
                    ins=[cc_in[:].opt()],
                    outs=[cc_out[:].opt()],
                )
                total = work.tile([1, BG_MAX], F32, tag="total")
                nc.sync.dma_start(total[:, :bg], cc_out[:])
                nc.sync.dma_start(stats_d[0:1, g0 : g0 + bg], total[:, :bg])

                recip = work.tile([1, BG_MAX], F32, tag="recip")
                nc.vector.reciprocal(recip[:, :bg], total[:, :bg])

                # broadcast recip to all 128 partitions via K=1 ones matmul
                psb = psbc.tile([P, BG_MAX], F32)
                nc.tensor.matmul(
                    psb[:, :bg], ones_k1[:], recip[:, :bg], start=True, stop=True
                )
                bcast = work.tile([P, BG_MAX], F32, tag="bcast", bufs=1)
                nc.vector.tensor_copy(bcast[:, :bg], psb[:, :bg])

                # normalize + write prediction shard
                p2eng = nc.vector if gi == len(GROUPS) - 1 else nc.gpsimd
                for t in range(NT):
                    e_t = e_tiles[t]
                    p2eng.tensor_mul(e_t[:, :bg], e_t[:, :bg], bcast[:, :bg])
                    nc.sync.dma_start(
                        out_d[t * P : (t + 1) * P, g0 : g0 + bg], e_t[:, :bg]
                    )

    nc.compile()
    return nc


def kernel(x, W, b, labels):
    x = np.ascontiguousarray(np.asarray(x), dtype=np.float32)
    W = np.ascontiguousarray(np.asarray(W), dtype=np.float32)
    b = np.ascontiguousarray(np.asarray(b), dtype=np.float32)
    labels_np = np.asarray(labels)

    if "nc" not in _CACHE:
        _CACHE["nc"] = _build()
    nc = _CACHE["nc"]

    xt_full = np.ascontiguousarray(x.T)  # [D, B]
    in_maps = []
    for ci in range(NC):
        w_sh = W[ci * CL : (ci + 1) * CL]  # [12500, 512]
        w_pad = np.zeros((CLP, D), dtype=np.float32)
        w_pad[:CL] = w_sh
        b_pad = np.full((CLP,), B_PAD, dtype=np.float32)
        b_pad[:CL] = b[ci * CL : (ci + 1) * CL]
        in_maps.append(
            {
                "xt": xt_full,
                "wt": np.ascontiguousarray(w_pad.T),  # [512, 12544]
                "wn": w_pad,
                "bv": b_pad,
            }
        )

    res = run_bass_kernel_spmd(nc, in_maps, core_ids=list(range(NC)))
    results = res.results

    # gather prediction: per-core [CLP, B] -> [B, C]
    pred_t = np.concatenate([results[ci]["out"][:CL] for ci in range(NC)], axis=0)
    pred = np.ascontiguousarray(pred_t.T)  # [1024, 100000] f32

    # host-side O(B) loss assembly (f32, IEEE inf-consistent)
    totals = results[0]["stats"][0].astype(np.float32)  # [B] softmax denominators
    # device sums are exp(S*wf - EXP_SHIFT); rescale in f64 then cast to f32 so
    # the cast overflows to inf exactly when the reference's f32 sum would
    e20_scaled = np.sum(
        np.stack([results[ci]["stats"][1] for ci in range(NC)]),
        axis=0,
        dtype=np.float32,
    )
    e20_total = (e20_scaled.astype(np.float64) * np.exp(EXP_SHIFT)).astype(
        np.float32
    )  # [B] sum_j exp(S*wf[i,j])

    lab = labels_np.astype(np.int64)
    tgt = pred[np.arange(B), lab]  # pred[i, y_i]
    # reconstruct wf[i, y_i] = log(pred * total); error ~1e-7 abs
    wf_t64 = np.log(tgt.astype(np.float64) * totals.astype(np.float64))
    wf_t = wf_t64.astype(np.float32)

    c = np.clip(wf_t, -1.0 + EPS, 1.0 - EPS)
    numerator = (S * np.cos(np.arccos(c.astype(np.float64)) + MARGIN)).astype(
        np.float32
    )
    target_exp = np.exp(S * wf_t64).astype(np.float32)  # inf iff f32 overflow

    excl = e20_total - target_exp
    L = numerator - np.log(np.exp(numerator) + excl)
    loss = np.float32(-np.mean(L))

    return pred, loss


# revision 11
# speedup vs baseline: 1.0125x; 1.0125x over previous
"""ArcFace loss_fn kernel for Trainium2, 8 NeuronCores.

Problem (hardcoded shapes): x [1024, 512] f32, W [100000, 512] f32,
b [100000] f32, labels [1024] int. Returns (prediction [1024, 100000] f32,
loss scalar f32) matching:

    Wn = W / max(||W_row||, 1e-12)
    wf = x @ Wn.T + b
    prediction = softmax(wf, axis=1)
    wf_m = cos(arccos(clip(wf, -1+eps, 1-eps)) + 0.1)
    numerator = 20 * wf_m[i, y_i]
    excl = sum_j exp(20*wf[i,j]) - exp(20*wf[i,y_i])
    loss = -mean(numerator - log(exp(numerator) + excl))

Sharding: class dim C split over 8 cores (12500 classes/core, zero-padded to
12544 = 98*128). Each core holds W.T + W shards, computes local logits via f32
TensorE matmuls, exp via ScalarE (fused scale=1/||W||, bias=b per-partition),
class-sums via DVE accumulate + ones-matmul partition fold, one small
AllReduce per batch-group for the global softmax denominator, and writes its
normalized prediction shard. The O(B)-sized loss assembly (target gather via
pred[i, y_i], cos/arccos, final log) runs on host in f32 — all O(B*C) work is
on device.
"""

import numpy as np

import concourse.bass as bass
import concourse.mybir as mybir
import concourse.tile as tile
from concourse import bacc
from concourse.bass_utils import run_bass_kernel_spmd

# problem constants
B = 1024
D = 512
C = 100000
S = 20.0
MARGIN = 0.1
EPS = 1e-7
NC = 8

CL = C // NC          # 12500 classes per core
P = 128
NT = 98               # C-tiles per core (12544 / 128)
CLP = NT * P          # 12544 padded classes per core
KT = D // P           # 4 contraction slices
B_PAD = -10000.0      # bias for padded classes -> exp(...) == 0
EXP_SHIFT = 100.0     # device computes exp(S*wf - EXP_SHIFT) so sums stay finite;
                      # host rescales by e^EXP_SHIFT in f32 to recover IEEE inf

# batch groups: E shard for one group must fit in SBUF alongside W stream
GROUPS = [(0, 344), (344, 340), (684, 340)]
BG_MAX = 344

F32 = mybir.dt.float32
BF16 = mybir.dt.bfloat16
SLABW = 512           # classes per W slab; hi|lo packed as [128, 2*SLABW] bf16
NSLAB = (CLP + SLABW - 1) // SLABW
AF = mybir.ActivationFunctionType
ALU = mybir.AluOpType

_CACHE = {}


def _build():
    nc = bacc.Bacc("TRN2", target_bir_lowering=False, debug=False, num_devices=NC)

    xt_d = nc.dram_tensor("xt", [D, B], F32, kind="ExternalInput").ap()
    wt_d = nc.dram_tensor("wt", [D, CLP], F32, kind="ExternalInput").ap()
    wn_d = nc.dram_tensor("wn", [CLP, D], F32, kind="ExternalInput").ap()
    b_d = nc.dram_tensor("bv", [CLP], F32, kind="ExternalInput").ap()
    out_d = nc.dram_tensor("out", [CLP, B], F32, kind="ExternalOutput").ap()
    stats_d = nc.dram_tensor("stats", [2, B], F32, kind="ExternalOutput").ap()

    with tile.TileContext(nc) as tc:
        with (
            tc.tile_pool(name="persist", bufs=1) as persist,
            tc.tile_pool(name="ebuf", bufs=1) as ebuf,
            tc.tile_pool(name="wtp", bufs=2) as wtp,
            tc.tile_pool(name="wtfp", bufs=4) as wtfp,
            tc.tile_pool(name="wnp", bufs=2) as wnp,
            tc.tile_pool(name="work", bufs=1) as work,
            tc.tile_pool(name="e20p", bufs=2) as e20p,
            tc.tile_pool(name="psmm", bufs=4, space="PSUM") as psmm,
            tc.tile_pool(name="psfold", bufs=1, space="PSUM") as psfold,
            tc.tile_pool(name="psbc", bufs=1, space="PSUM") as psbc,
            tc.tile_pool(name="psdump", bufs=1, space="PSUM") as psdump,
            tc.tile_pool(name="dram", bufs=1, space="DRAM") as dram,
        ):
            # ---- resident small tensors ----
            # x split into bf16 hi + lo (x = hi + lo to ~2^-17 rel)
            xhi = [
                persist.tile([P, B], BF16, tag=f"xhi{k}", name=f"xhi{k}")
                for k in range(KT)
            ]
            xlo = [
                persist.tile([P, B], BF16, tag=f"xlo{k}", name=f"xlo{k}")
                for k in range(KT)
            ]
            for k in range(KT):
                xtf = wtfp.tile([P, B], F32, tag="xtf", name="xtf", bufs=1)
                nc.sync.dma_start(xtf[:], xt_d[k * P : (k + 1) * P, :])
                nc.vector.tensor_copy(xhi[k][:], xtf[:])
                nc.vector.scalar_tensor_tensor(
                    xlo[k][:], xhi[k][:], -1.0, xtf[:], ALU.mult, ALU.add
                )

            ones_p = persist.tile([P, 1], F32, tag="ones_p")
            nc.vector.memset(ones_p[:], 1.0)
            ones_bf = persist.tile([P, 1], BF16, tag="ones_bf")
            nc.vector.memset(ones_bf[:], 1.0)
            ones_k1 = persist.tile([1, P], F32, tag="ones_k1")
            nc.vector.memset(ones_k1[:], 1.0)

            b_all = persist.tile([P, NT], F32, tag="b_all")
            nc.sync.dma_start(b_all[:], b_d.rearrange("(t p) -> p t", p=P))
            b20 = persist.tile([P, NT], F32, tag="b20")
            nc.vector.tensor_scalar(b20[:], b_all[:], S, -EXP_SHIFT, ALU.mult, ALU.add)

            # DRAM spill for bf16 hi|lo W blocks, written during group 0
            wt_hl_d = dram.tile(
                [NSLAB, KT, P, 2 * SLABW], BF16, tag="wt_hl", name="wt_hl"
            )

            # ---- W row norms: ssq per class via Square + accum ----
            # chunked so early C-tiles' inv/inv20 unblock before the full pass
            ssq = persist.tile([P, NT], F32, tag="ssq")
            ssq_c = persist.tile([P, NT], F32, tag="ssq_c")
            inv = persist.tile([P, NT], F32, tag="inv")
            inv20 = persist.tile([P, NT], F32, tag="inv20")
            zz = persist.tile([P, NT], F32, tag="zz")
            nrm = persist.tile([P, NT], F32, tag="nrm")
            NCHUNK = 25
            for c0 in range(0, NT, NCHUNK):
                c1 = min(c0 + NCHUNK, NT)
                for t in range(c0, c1):
                    wn_t = wnp.tile([P, D], F32, tag="wn", name="wn_t")
                    nc.sync.dma_start(wn_t[:], wn_d[t * P : (t + 1) * P, :])
                    dump = psdump.tile([P, D], F32, tag="dump", name="dump")
                    nc.scalar.activation(
                        dump[:], wn_t[:], AF.Square, accum_out=ssq[:, t : t + 1]
                    )
                cs = slice(c0, c1)
                # inv = 1/sqrt(max(ssq,1e-24)), two Newton steps for f32 accuracy
                nc.vector.tensor_scalar_max(ssq_c[:, cs], ssq[:, cs], 1e-24)
                nc.scalar.activation(nrm[:, cs], ssq_c[:, cs], AF.Sqrt)
                nc.vector.reciprocal(inv[:, cs], nrm[:, cs])
                for _ in range(2):
                    nc.vector.tensor_mul(zz[:, cs], inv[:, cs], inv[:, cs])
                    nc.vector.tensor_mul(zz[:, cs], zz[:, cs], ssq_c[:, cs])
                    nc.vector.tensor_scalar(
                        zz[:, cs], zz[:, cs], -0.5, 1.5, ALU.mult, ALU.add
                    )
                    nc.vector.tensor_mul(inv[:, cs], inv[:, cs], zz[:, cs])
                nc.vector.tensor_scalar_mul(inv20[:, cs], inv[:, cs], S)

            # ---- main: per batch-group ----
            for gi, (g0, bg) in enumerate(GROUPS):
                accE = persist.tile([P, BG_MAX], F32, tag="accE")
                nc.vector.memset(accE[:, :bg], 0.0)
                # E20 row-sums accumulate on PE: bf16 ones-matmul into one PSUM
                # bank across all 98 C-tiles (loss stats need only ~1e-3)
                psE20 = psfold.tile([1, BG_MAX], F32, tag="psE20", name="psE20")

                e_tiles = []
                for si, t0 in enumerate(range(0, NT, 4)):
                    nts = min(4, NT - t0)
                    w = nts * P
                    wts = []
                    for k in range(KT):
                        hl = wtp.tile(
                            [P, 2 * SLABW], BF16, tag=f"wt{k}", name=f"wt{k}"
                        )
                        if gi == 0:
                            # split W.T slab into bf16 hi|lo and spill to DRAM
                            wtf = wtfp.tile([P, SLABW], F32, tag="wtf", name="wtf")
                            nc.sync.dma_start(
                                wtf[:, :w],
                                wt_d[k * P : (k + 1) * P, t0 * P : t0 * P + w],
                            )
                            nc.scalar.activation(hl[:, :w], wtf[:, :w], AF.Copy)
                            nc.vector.scalar_tensor_tensor(
                                hl[:, SLABW : SLABW + w],
                                hl[:, :w],
                                -1.0,
                                wtf[:, :w],
                                ALU.mult,
                                ALU.add,
                            )
                            nc.sync.dma_start(
                                wt_hl_d[si, k, :, : SLABW + w], hl[:, : SLABW + w]
                            )
                        else:
                            nc.sync.dma_start(hl[:], wt_hl_d[si, k, :, :])
                        wts.append(hl)
                    for j in range(nts):
                        t = t0 + j
                        ps = psmm.tile([P, BG_MAX], F32, name="ps")
                        mm = 0
                        for k in range(KT):
                            hi = wts[k][:, j * P : (j + 1) * P]
                            lo = wts[k][:, SLABW + j * P : SLABW + (j + 1) * P]
                            for wop, xop in ((hi, xhi[k]), (hi, xlo[k]), (lo, xhi[k])):
                                nc.tensor.matmul(
                                    ps[:, :bg],
                                    wop,
                                    xop[:, g0 : g0 + bg],
                                    start=(mm == 0),
                                    stop=(mm == 3 * KT - 1),
                                )
                                mm += 1
                        e_t = ebuf.tile(
                            [P, BG_MAX], F32, tag=f"e{t}", name=f"e{t}",
                            bufs=2 if t < 8 else 1,
                        )
                        nc.scalar.activation(
                            e_t[:, :bg],
                            ps[:, :bg],
                            AF.Exp,
                            bias=b_all[:, t : t + 1],
                            scale=inv[:, t : t + 1],
                        )
                        e20_t = e20p.tile([P, BG_MAX], BF16, tag="e20", name="e20")
                        nc.scalar.activation(
                            e20_t[:, :bg],
                            ps[:, :bg],
                            AF.Exp,
                            bias=b20[:, t : t + 1],
                            scale=inv20[:, t : t + 1],
                        )
                        nc.vector.tensor_add(accE[:, :bg], accE[:, :bg], e_t[:, :bg])
                        nc.tensor.matmul(
                            psE20[:, :bg],
                            ones_bf[:],
                            e20_t[:, :bg],
                            start=(t == 0),
                            stop=(t == NT - 1),
                        )
                        e_tiles.append(e_t)

                # fold partitions: [128, bg] -> [1, bg]
                psE = psfold.tile([1, BG_MAX], F32, tag="psE")
                nc.tensor.matmul(
                    psE[:, :bg], ones_p[:], accE[:, :bg], start=True, stop=True
                )
                sumE = work.tile([1, BG_MAX], F32, tag="sumE")
                nc.vector.tensor_copy(sumE[:, :bg], psE[:, :bg])

                sumE20 = work.tile([1, BG_MAX], F32, tag="sumE20")
                nc.vector.tensor_copy(sumE20[:, :bg], psE20[:, :bg])
                nc.sync.dma_start(stats_d[1:2, g0 : g0 + bg], sumE20[:, :bg])

                # AllReduce softmax denominator across the 8 cores
                cc_in = dram.tile([1, bg], F32, tag=f"ccin{gi}")
                cc_out = dram.tile([1, bg], F32, tag=f"ccout{gi}")
                nc.sync.dma_start(cc_in[:], sumE[:, :bg])
                nc.gpsimd.collective_compute(
                    "AllReduce",
                    ALU.add,
                    replica_groups=[list(range(NC))],
                    ins=[cc_in[:].opt()],
                    outs=[cc_out[:].opt()],
                )
                total = work.tile([1, BG_MAX], F32, tag="total")
                nc.sync.dma_start(total[:, :bg], cc_out[:])
                nc.sync.dma_start(stats_d[0:1, g0 : g0 + bg], total[:, :bg])

                recip = work.tile([1, BG_MAX], F32, tag="recip")
                nc.vector.reciprocal(recip[:, :bg], total[:, :bg])

                # broadcast recip to all 128 partitions via K=1 ones matmul
                psb = psbc.tile([P, BG_MAX], F32)
                nc.tensor.matmul(
                    psb[:, :bg], ones_k1[:], recip[:, :bg], start=True, stop=True
                )
                bcast = work.tile([P, BG_MAX], F32, tag="bcast", bufs=1)
                nc.vector.tensor_copy(bcast[:, :bg], psb[:, :bg])

                # normalize + write prediction shard
                for t in range(NT):
                    e_t = e_tiles[t]
                    p2eng = nc.vector if t % 2 == 0 else nc.gpsimd
                    p2eng.tensor_mul(e_t[:, :bg], e_t[:, :bg], bcast[:, :bg])
                    nc.sync.dma_start(
                        out_d[t * P : (t + 1) * P, g0 : g0 + bg], e_t[:, :bg]
                    )

    nc.compile()
    return nc


def kernel(x, W, b, labels):
    x = np.ascontiguousarray(np.asarray(x), dtype=np.float32)
    W = np.ascontiguousarray(np.asarray(W), dtype=np.float32)
    b = np.ascontiguousarray(np.asarray(b), dtype=np.float32)
    labels_np = np.asarray(labels)

    if "nc" not in _CACHE:
        _CACHE["nc"] = _build()
    nc = _CACHE["nc"]

    xt_full = np.ascontiguousarray(x.T)  # [D, B]
    in_maps = []
    for ci in range(NC):
        w_sh = W[ci * CL : (ci + 1) * CL]  # [12500, 512]
        w_pad = np.zeros((CLP, D), dtype=np.float32)
        w_pad[:CL] = w_sh
        b_pad = np.full((CLP,), B_PAD, dtype=np.float32)
        b_pad[:CL] = b[ci * CL : (ci + 1) * CL]
        in_maps.append(
            {
                "xt": xt_full,
                "wt": np.ascontiguousarray(w_pad.T),  # [512, 12544]
                "wn": w_pad,
                "bv": b_pad,
            }
        )

    res = run_bass_kernel_spmd(nc, in_maps, core_ids=list(range(NC)))
    results = res.results

    # gather prediction: per-core [CLP, B] -> [B, C]
    pred_t = np.concatenate([results[ci]["out"][:CL] for ci in range(NC)], axis=0)
    pred = np.ascontiguousarray(pred_t.T)  # [1024, 100000] f32

    # host-side O(B) loss assembly (f32, IEEE inf-consistent)
    totals = results[0]["stats"][0].astype(np.float32)  # [B] softmax denominators
    # device sums are exp(S*wf - EXP_SHIFT); rescale in f64 then cast to f32 so
    # the cast overflows to inf exactly when the reference's f32 sum would
    e20_scaled = np.sum(
        np.stack([results[ci]["stats"][1] for ci in range(NC)]),
        axis=0,
        dtype=np.float32,
    )
    e20_total = (e20_scaled.astype(np.float64) * np.exp(EXP_SHIFT)).astype(
        np.float32
    )  # [B] sum_j exp(S*wf[i,j])

    lab = labels_np.astype(np.int64)
    tgt = pred[np.arange(B), lab]  # pred[i, y_i]
    # reconstruct wf[i, y_i] = log(pred * total); error ~1e-7 abs
    wf_t64 = np.log(tgt.astype(np.float64) * totals.astype(np.float64))
    wf_t = wf_t64.astype(np.float32)

    c = np.clip(wf_t, -1.0 + EPS, 1.0 - EPS)
    numerator = (S * np.cos(np.arccos(c.astype(np.float64)) + MARGIN)).astype(
        np.float32
    )
    target_exp = np.exp(S * wf_t64).astype(np.float32)  # inf iff f32 overflow

    excl = e20_total - target_exp
    L = numerator - np.log(np.exp(numerator) + excl)
    loss = np.float32(-np.mean(L))

    return pred, loss
